# revision 45
# baseline (speedup 1.0000x reference)
"""GCN 2-layer forward on 8 Trainium2 NeuronCores.

Strategy (dst-sharded, feature-major, ap_gather ELL):
- Nodes degree-sorted; global slot s -> core s%8, local slot j=s//8 (12500
  real, padded to 12544 per core).
- Each core owns all in-edges of its nodes. Gather tables (y = dinv * xW)
  are feature-major [16, 12544] per core, all-gathered so every core holds
  all 8 chunks; chunk g lives on SBUF partitions 16g..16g+15 (f32, d=1).
- Self-loops are NOT routed through the gather: their contribution
  dinv[dst]*y[dst] is accumulated in the merge matmul via a per-core
  selector matrix over the core's own tab chunk (canonical order). This
  keeps the 8 edge groups balanced (~50K edges each) and cuts ELL padding.
- Edges are routed to GPSIMD group g = owner-core-of-src. Each group
  accumulates partials for ALL of the core's slots in its own private
  order (slots sorted by that group's realized edge count -> exact ELL
  round prefixes).
- ap_gather gathers message streams (rounds, zero-slot padded); DVE adds
  accumulate round prefixes into acc [128, 12544].
- A second small ap_gather canonicalizes each group's acc into the shared
  local-slot order; a PE matmul with a 0/1 selection matrix sums the 8
  groups; epilogue (own-chunk add, dinv scale, bias, relu, W2, W_lin)
  runs feature-major in 448-wide pieces on PE/DVE/ACT.
- dinv (deg^-1/2, 0 on padded slots) is precomputed on host as [16,12544].
"""
import sys
sys.path.insert(0, "/opt/trn_rl_repo")
import numpy as np

N_NODES = 100000
N_EDGES = 3200000
D_IN = 128
H = 16
CORES = 8
LOC = 12544          # padded local slots per core (12500 real)
REAL = 12500
ZERO_SLOT = 12500    # any padded local slot: y value is 0 there
CALL = 1792          # idxs per ap_gather call (= 4 * PIECE)
PIECE = 448          # matmul moving width (LOC = 28 * 448)


# ---------------------------------------------------------------- host prep
def _ceil16(x):
    return -(-x // 16) * 16


def host_prep(x, edge_index):
    src = edge_index[0].astype(np.int64)
    dst = edge_index[1].astype(np.int64)

    # degree INCLUDES self-loops (reference adds them), even though the
    # self-loop messages themselves are handled outside the gather.
    deg = np.bincount(dst, minlength=N_NODES) + 1
    perm = np.argsort(-deg, kind="stable")
    rank = np.empty(N_NODES, np.int64)
    rank[perm] = np.arange(N_NODES)

    s_dst = rank[dst]
    s_src = rank[src]
    core_e = s_dst % CORES
    dloc_e = s_dst // CORES
    g_e = (s_src % CORES).astype(np.int64)
    sloc_e = s_src // CORES

    # per (core k, group g): counts, private order, sorted edge lists
    per = {}
    Lmax = {}
    Rglob = 0
    for k in range(CORES):
        mk = core_e == k
        for g in range(CORES):
            m = mk & (g_e == g)
            dl = dloc_e[m]
            sl = sloc_e[m]
            cnt = np.bincount(dl, minlength=LOC)
            order = np.argsort(-cnt, kind="stable")       # group pos -> dloc
            pos_of = np.empty(LOC, np.int64)
            pos_of[order] = np.arange(LOC)
            o = np.argsort(dl, kind="stable")
            sl_sorted = sl[o]
            starts = np.zeros(LOC + 1, np.int64)
            starts[1:] = np.cumsum(cnt)
            R = int(cnt.max())
            Rglob = max(Rglob, R)
            per[(k, g)] = (cnt, order, pos_of, sl_sorted, starts)
            cnt_sorted = cnt[order]
            for r in range(1, R + 1):
                nz = np.nonzero(cnt_sorted >= r)[0]
                L = int(nz[-1]) + 1 if nz.size else 0
                Lmax[r] = max(Lmax.get(r, 0), L)

    L16 = [_ceil16(Lmax[r]) for r in range(1, Rglob + 1)]
    offs = np.concatenate([[0], np.cumsum(L16)]).astype(np.int64)
    TOT = int(offs[-1])

    # calls and add-segments (shared structure)
    n_call = -(-TOT // CALL)
    call_len = [min(CALL, TOT - c * CALL) for c in range(n_call)]
    segments = []  # (call, dest_off, acc_off, length)
    for r in range(Rglob):
        a, b = int(offs[r]), int(offs[r + 1])
        p = a
        while p < b:
            c = p // CALL
            e = min(b, (c + 1) * CALL)
            segments.append((c, p - c * CALL, p - a, e - p))
            p = e

    # per-core idx tensors
    def wrap16(flat):
        n = flat.size
        return flat.reshape(n // 16, 16).T

    IDX = np.full((CORES, 128, TOT // 16), ZERO_SLOT, np.int16)
    IDXC = np.zeros((CORES, 128, LOC // 16), np.int16)
    for k in range(CORES):
        for g in range(CORES):
            cnt, order, pos_of, sl_sorted, starts = per[(k, g)]
            stream = np.full(TOT, ZERO_SLOT, np.int64)
            for r in range(1, Rglob + 1):
                a = int(offs[r - 1])
                L = L16[r - 1]
                d_arr = order[:L]
                valid = cnt[d_arr] >= r
                pos = np.clip(starts[d_arr] + r - 1, 0, sl_sorted.size - 1)
                vals = np.where(valid, sl_sorted[pos] if sl_sorted.size else 0,
                                ZERO_SLOT)
                stream[a : a + L] = vals
            IDX[k, 16 * g : 16 * g + 16, :] = wrap16(stream)
            IDXC[k, 16 * g : 16 * g + 16, :] = wrap16(pos_of)

    # per-core x.T and dinv
    XT = np.zeros((CORES, 128, LOC), np.float32)
    DINV = np.zeros((CORES, 16, LOC), np.float32)
    node_of = np.zeros((CORES, REAL), np.int64)
    dinv_all = 1.0 / np.sqrt(deg.astype(np.float64))
    for k in range(CORES):
        nodes = perm[k::CORES]           # local j -> node
        node_of[k] = nodes
        XT[k, :, :REAL] = x[nodes].T
        DINV[k, :, :REAL] = dinv_all[nodes][None, :].astype(np.float32)

    SMERGE = np.zeros((128, 16), np.float32)
    for g in range(CORES):
        for f in range(16):
            SMERGE[16 * g + f, f] = 1.0
    # per-core selector for the self-loop term: pick the core's own chunk
    # (partitions 16k..16k+15 of tab, already in canonical slot order)
    SMK = np.zeros((CORES, 128, 16), np.float32)
    for k in range(CORES):
        for f in range(16):
            SMK[k, 16 * k + f, f] = 1.0

    struct = dict(Rglob=Rglob, L16=L16, TOT=TOT, n_call=n_call,
                  call_len=call_len, segments=segments)
    return struct, IDX, IDXC, XT, DINV, SMERGE, SMK, node_of


# ---------------------------------------------------------------- device build
def build_nc(struct):
    import concourse.bass as bass
    import concourse.bacc as bacc
    import concourse.mybir as mybir
    import concourse.tile as tile

    f32 = mybir.dt.float32
    TOT, n_call = struct["TOT"], struct["n_call"]
    call_len, segments = struct["call_len"], struct["segments"]

    nc = bacc.Bacc(None, target_bir_lowering=False)
    t_xt = nc.dram_tensor("xt", [128, LOC], f32, kind="ExternalInput")
    t_idx = nc.dram_tensor("idx", [128, TOT // 16], mybir.dt.int16,
                           kind="ExternalInput")
    t_idxc = nc.dram_tensor("idxc", [128, LOC // 16], mybir.dt.int16,
                            kind="ExternalInput")
    t_dinv = nc.dram_tensor("dinv", [16, LOC], f32, kind="ExternalInput")
    t_w1 = nc.dram_tensor("w1", [128, H], f32, kind="ExternalInput")
    t_w2 = nc.dram_tensor("w2", [H, H], f32, kind="ExternalInput")
    t_wl = nc.dram_tensor("wl", [H, 1], f32, kind="ExternalInput")
    t_b1 = nc.dram_tensor("b1", [H, 1], f32, kind="ExternalInput")
    t_b2 = nc.dram_tensor("b2", [H, 1], f32, kind="ExternalInput")
    t_sm = nc.dram_tensor("sm", [128, H], f32, kind="ExternalInput")
    t_smk = nc.dram_tensor("smk", [128, H], f32, kind="ExternalInput")
    t_out = nc.dram_tensor("out", [1, LOC], f32, kind="ExternalOutput")
    # b_lin is 0 in the reference; fold as constant 0 (skip).

    n_piece = LOC // PIECE
    assert LOC % PIECE == 0

    with tile.TileContext(nc) as tc:
        with (
            tc.tile_pool(name="sbuf", bufs=1) as pool,
            tc.tile_pool(name="io", bufs=3) as iop,
            tc.tile_pool(name="io2", bufs=6) as iop2,
            tc.tile_pool(name="psum", bufs=2, space="PSUM") as pp,
            tc.tile_pool(name="psum2", bufs=4, space="PSUM") as pp2,
            tc.tile_pool(name="dram", bufs=1, space="DRAM") as dram,
        ):
            # persistent tiles
            idxt = pool.tile([128, TOT // 16], mybir.dt.int16, name="idxt")
            idxct = pool.tile([128, LOC // 16], mybir.dt.int16, name="idxct")
            tab = pool.tile([128, LOC], f32, name="tab")
            acc = pool.tile([128, LOC], f32, name="acc")
            w1t = pool.tile([128, H], f32, name="w1t")
            w2t = pool.tile([H, H], f32, name="w2t")
            wlt = pool.tile([H, 1], f32, name="wlt")
            b1t = pool.tile([H, 1], f32, name="b1t")
            b2t = pool.tile([H, 1], f32, name="b2t")
            smt = pool.tile([128, H], f32, name="smt")
            smkt = pool.tile([128, H], f32, name="smkt")

            nc.sync.dma_start(out=idxt[:], in_=t_idx[:, :])
            nc.sync.dma_start(out=idxct[:], in_=t_idxc[:, :])
            nc.sync.dma_start(out=w1t[:], in_=t_w1[:, :])
            nc.sync.dma_start(out=w2t[:], in_=t_w2[:, :])
            nc.sync.dma_start(out=wlt[:], in_=t_wl[:, :])
            nc.sync.dma_start(out=b1t[:], in_=t_b1[:, :])
            nc.sync.dma_start(out=b2t[:], in_=t_b2[:, :])
            nc.sync.dma_start(out=smt[:], in_=t_sm[:, :])
            nc.sync.dma_start(out=smkt[:], in_=t_smk[:, :])

            HALF = 14 * PIECE            # 6272: AllGather split point
            ytab_full = []
            for layer in range(2):
                bounce_a = dram.tile([16, HALF], f32, tag=f"bna{layer}",
                                     name=f"bna{layer}")
                bounce_b = dram.tile([16, LOC - HALF], f32,
                                     tag=f"bnb{layer}", name=f"bnb{layer}")
                full_a = nc.dram_tensor(f"full{layer}a", [CORES, 16, HALF],
                                        f32, addr_space="Shared")
                full_b = nc.dram_tensor(f"full{layer}b", [CORES, 16,
                                        LOC - HALF], f32, addr_space="Shared")
                ytab_full.append((bounce_a, bounce_b, full_a, full_b))

            def bounce_slice(layer, off):
                # piece destination within the layer's split bounce pair
                bounce_a, bounce_b = ytab_full[layer][:2]
                if off < HALF:
                    return bounce_a[:, off : off + PIECE]
                return bounce_b[:, off - HALF : off - HALF + PIECE]

            def ag_half(layer, half):
                # the collective is a non-blocking issue on POOL (transfer
                # runs on the CC DMA); splitting lets the first half's
                # transfer overlap the compute producing the second half
                bounce_a, bounce_b, full_a, full_b = ytab_full[layer]
                if half == 0:
                    ins, outs = bounce_a[:], full_a.ap()
                else:
                    ins, outs = bounce_b[:], full_b.ap()
                nc.gpsimd.collective_compute(
                    "AllGather", mybir.AluOpType.bypass,
                    replica_groups=[list(range(CORES))],
                    ins=[ins.opt()], outs=[outs.opt()])

            def load_tab(layer):
                full_a, full_b = ytab_full[layer][2:]
                for g in range(CORES):
                    nc.sync.dma_start(out=tab[16 * g : 16 * g + 16, :HALF],
                                      in_=full_a[g, :, :])
                    nc.sync.dma_start(out=tab[16 * g : 16 * g + 16, HALF:],
                                      in_=full_b[g, :, :])

            CH = 2 * PIECE               # 896: xT/dinv load chunk

            def build_table_layer0():
                # y1 = dinv * (W1.T @ xT), chunked loads, piece matmuls
                for c in range(LOC // CH):
                    base = c * CH
                    xin = iop2.tile([128, CH], f32, tag="xin")
                    nc.sync.dma_start(out=xin[:],
                                      in_=t_xt[:, base : base + CH])
                    dv = iop2.tile([16, CH], f32, tag="dvb")
                    nc.sync.dma_start(out=dv[:],
                                      in_=t_dinv[:, base : base + CH])
                    for q in range(CH // PIECE):
                        a = base + q * PIECE
                        ps = pp2.tile([16, PIECE], f32, tag="ps")
                        nc.tensor.matmul(
                            out=ps[:], lhsT=w1t[:],
                            rhs=xin[:, q * PIECE : (q + 1) * PIECE],
                            start=True, stop=True)
                        yp = iop.tile([16, PIECE], f32, tag="ep")
                        nc.vector.tensor_mul(
                            out=yp[:], in0=ps[:],
                            in1=dv[:, q * PIECE : (q + 1) * PIECE])
                        nc.sync.dma_start(out=bounce_slice(0, a), in_=yp[:])
                        if a + PIECE == HALF:
                            ag_half(0, 0)
                ag_half(0, 1)

            def gather_accumulate():
                nc.vector.memset(acc[:], 0.0)
                for c in range(n_call):
                    ln = call_len[c]
                    d = iop.tile([128, CALL], f32, tag="gd")
                    nc.gpsimd.ap_gather(
                        d[:, :ln], tab[:],
                        idxt[:, c * (CALL // 16) : c * (CALL // 16) + ln // 16],
                        channels=128, num_elems=LOC, d=1, num_idxs=ln)
                    for (cc, doff, aoff, ln2) in segments:
                        if cc != c:
                            continue
                        nc.vector.tensor_add(
                            out=acc[:, aoff : aoff + ln2],
                            in0=acc[:, aoff : aoff + ln2],
                            in1=d[:, doff : doff + ln2])

            def canonicalize_and_epilogue(layer):
                n_cc = LOC // CALL + (1 if LOC % CALL else 0)
                for c in range(n_cc):
                    a = c * CALL
                    b = min(LOC, a + CALL)
                    w = b - a
                    cd = iop.tile([128, CALL], f32, tag="gd")
                    nc.gpsimd.ap_gather(
                        cd[:, :w], acc[:],
                        idxct[:, a // 16 : b // 16],
                        channels=128, num_elems=LOC, d=1, num_idxs=w)
                    dv = iop.tile([16, CALL], f32, tag="dvc")
                    nc.sync.dma_start(out=dv[:, :w], in_=t_dinv[:, a:b])
                    for q in range(w // PIECE):
                        off = a + q * PIECE
                        ps = pp2.tile([16, PIECE], f32, tag="ps")
                        # sum the 8 group partials, then accumulate the
                        # self-loop term (core's own tab chunk) in PSUM
                        nc.tensor.matmul(out=ps[:], lhsT=smt[:],
                                         rhs=cd[:, q * PIECE : (q + 1) * PIECE],
                                         start=True, stop=False)
                        nc.tensor.matmul(out=ps[:], lhsT=smkt[:],
                                         rhs=tab[:, off : off + PIECE],
                                         start=False, stop=True)
                        v = iop.tile([16, PIECE], f32, tag="ep")
                        nc.vector.tensor_mul(
                            out=v[:], in0=ps[:],
                            in1=dv[:, q * PIECE : q * PIECE + PIECE])
                        if layer == 0:
                            # y2 = dinv * relu(v + b1) -> bounce1
                            h = iop.tile([16, PIECE], f32, tag="ep")
                            nc.scalar.activation(
                                out=h[:], in_=v[:],
                                func=mybir.ActivationFunctionType.Relu,
                                bias=b1t[:])
                            y2 = iop.tile([16, PIECE], f32, tag="ep")
                            nc.vector.tensor_mul(
                                out=y2[:], in0=h[:],
                                in1=dv[:, q * PIECE : q * PIECE + PIECE])
                            nc.sync.dma_start(
                                out=bounce_slice(1, off), in_=y2[:])
                            if off + PIECE == HALF:
                                ag_half(1, 0)
                        else:
                            # z = W2.T @ v ; h2 = relu(z + b2); o = Wl.T @ h2
                            ps2 = pp.tile([16, PIECE], f32, tag="ps2")
                            nc.tensor.matmul(out=ps2[:], lhsT=w2t[:],
                                             rhs=v[:], start=True, stop=True)
                            h2 = iop.tile([16, PIECE], f32, tag="ep")
                            nc.scalar.activation(
                                out=h2[:], in_=ps2[:],
                                func=mybir.ActivationFunctionType.Relu,
                                bias=b2t[:])
                            ps3 = pp.tile([1, PIECE], f32, tag="ps3")
                            nc.tensor.matmul(out=ps3[:], lhsT=wlt[:],
                                             rhs=h2[:], start=True, stop=True)
                            ob = iop.tile([1, PIECE], f32, tag="ep")
                            nc.vector.tensor_copy(out=ob[:], in_=ps3[:])
                            nc.sync.dma_start(
                                out=t_out[:, off : off + PIECE], in_=ob[:])

            build_table_layer0()
            load_tab(0)
            gather_accumulate()
            canonicalize_and_epilogue(0)
            ag_half(1, 1)
            load_tab(1)
            gather_accumulate()
            canonicalize_and_epilogue(1)

    nc.finalize()
    return nc


# ---------------------------------------------------------------- runner
class _Runner:
    def __init__(self, nc, n_cores):
        import jax
        import numpy as _np
        from jax.sharding import Mesh, PartitionSpec, NamedSharding
        from jax.experimental.shard_map import shard_map
        import concourse.mybir as mybir
        from concourse.bass2jax import (
            _bass_exec_p, install_neuronx_cc_hook, partition_id_tensor)

        install_neuronx_cc_hook()
        self.nc = nc
        self.n_cores = n_cores
        partition_name = (nc.partition_id_tensor.name
                          if nc.partition_id_tensor else None)
        in_names, out_names, out_avals, zero_outs = [], [], [], []
        for alloc in nc.m.functions[0].allocations:
            if not isinstance(alloc, mybir.MemoryLocationSet):
                continue
            name = alloc.memorylocations[0].name
            if alloc.kind == "ExternalInput":
                if name != partition_name:
                    in_names.append(name)
            elif alloc.kind == "ExternalOutput":
                shape = tuple(alloc.tensor_shape)
                dtype = mybir.dt.np(alloc.dtype)
                out_names.append(name)
                out_avals.append(jax.core.ShapedArray(shape, dtype))
                zero_outs.append(_np.zeros(shape, dtype))
        self.in_names, self.out_names = in_names, out_names
        self.out_avals, self.zero_outs = out_avals, zero_outs
        n_params, n_outs = len(in_names), len(out_avals)
        all_in = in_names + out_names
        if partition_name is not None:
            all_in.append(partition_name)
        donate = tuple(range(n_params, n_params + n_outs))

        def _body(*args):
            operands = list(args)
            if partition_name is not None:
                operands.append(partition_id_tensor())
            return tuple(_bass_exec_p.bind(
                *operands, out_avals=tuple(out_avals),
                in_names=tuple(all_in), out_names=tuple(out_names),
                lowering_input_output_aliases=(),
                sim_require_finite=True, sim_require_nnan=True, nc=nc))

        devices = jax.devices()[:n_cores]
        mesh = Mesh(_np.asarray(devices), ("core",))
        self._sharding = NamedSharding(mesh, PartitionSpec("core"))
        in_specs = (PartitionSpec("core"),) * (n_params + n_outs)
        out_specs = (PartitionSpec("core"),) * len(out_names)
        self._fn = jax.jit(
            shard_map(_body, mesh=mesh, in_specs=in_specs,
                      out_specs=out_specs, check_rep=False),
            donate_argnums=donate, keep_unused=True)

        # device-side zero buffers for the donated outputs: generated on
        # device each call so repeats never pay H2D for them.
        import jax.numpy as jnp
        zero_shapes = [(n_cores * z.shape[0], *z.shape[1:])
                       for z in self.zero_outs]
        zero_dtypes = [z.dtype for z in self.zero_outs]

        def _mk_zeros():
            return tuple(jnp.zeros(s, d) for s, d in
                         zip(zero_shapes, zero_dtypes))

        self._mk_zeros = jax.jit(
            _mk_zeros, out_shardings=tuple([self._sharding] * n_outs))
        # cache of device-resident concatenated inputs (keyed on in_maps id)
        self._dev_key = None
        self._dev_in = None
        # previous call's output buffers, recycled as the next call's
        # donated output arguments (the kernel writes every element of
        # each output, so stale contents are harmless)
        self._recycle = None

    def _device_inputs(self, in_maps):
        import jax
        import numpy as _np
        key = id(in_maps)
        if self._dev_key == key and self._dev_in is not None:
            return self._dev_in
        n = self.n_cores
        per_core = [[_np.asarray(m[name]) for name in self.in_names]
                    for m in in_maps]
        concat_in = [
            _np.concatenate([per_core[c][i] for c in range(n)], axis=0)
            for i in range(len(self.in_names))]
        self._dev_in = [jax.device_put(a, self._sharding) for a in concat_in]
        self._dev_key = key
        return self._dev_in

    def __call__(self, in_maps):
        import numpy as _np
        n = self.n_cores
        dev_in = self._device_inputs(in_maps)
        donated = self._recycle if self._recycle is not None \
            else self._mk_zeros()
        outs = self._fn(*dev_in, *donated)
        out_arrs = [_np.asarray(a) for a in outs]
        self._recycle = outs
        return [
            {name: out_arrs[i].reshape(n, *self.out_avals[i].shape)[c]
             for i, name in enumerate(self.out_names)}
            for c in range(n)]


_CACHE = {}


def kernel(x, edge_index, W1, b1, W2, b2, W_lin, b_lin):
    x = np.asarray(x, np.float32)
    edge_index = np.asarray(edge_index)
    struct, IDX, IDXC, XT, DINV, SMERGE, SMK, node_of = host_prep(
        x, edge_index)

    key = repr(sorted(struct.items()))
    if key not in _CACHE:
        nc = build_nc(struct)
        _CACHE[key] = _Runner(nc, CORES)
    runner = _CACHE[key]

    in_maps = []
    for k in range(CORES):
        in_maps.append({
            "xt": XT[k], "idx": IDX[k], "idxc": IDXC[k], "dinv": DINV[k],
            "w1": np.asarray(W1, np.float32),
            "w2": np.asarray(W2, np.float32),
            "wl": np.asarray(W_lin, np.float32),
            "b1": np.asarray(b1, np.float32).reshape(H, 1),
            "b2": np.asarray(b2, np.float32).reshape(H, 1),
            "sm": SMERGE,
            "smk": SMK[k],
        })
    res = runner(in_maps)
    out = np.zeros(N_NODES, np.float32)
    blin = float(np.asarray(b_lin).reshape(-1)[0])
    for k in range(CORES):
        out[node_of[k]] = res[k]["out"][0, :REAL] + blin
    kernel.last_runner = runner
    kernel.last_in_maps = in_maps
    return out


# revision 46
# speedup vs baseline: 1.0086x; 1.0086x over previous
"""GCN 2-layer forward on 8 Trainium2 NeuronCores.

Strategy (dst-sharded, feature-major, ap_gather ELL):
- Nodes degree-sorted; global slot s -> core s%8, local slot j=s//8 (12500
  real, padded to 12544 per core).
- Each core owns all in-edges of its nodes. Gather tables (y = dinv * xW)
  are feature-major [16, 12544] per core, all-gathered so every core holds
  all 8 chunks; chunk g lives on SBUF partitions 16g..16g+15 (f32, d=1).
- Self-loops are NOT routed through the gather: their contribution
  dinv[dst]*y[dst] is accumulated in the merge matmul via a per-core
  selector matrix over the core's own tab chunk (canonical order). This
  keeps the 8 edge groups balanced (~50K edges each) and cuts ELL padding.
- Edges are routed to GPSIMD group g = owner-core-of-src. Each group
  accumulates partials for ALL of the core's slots in its own private
  order (slots sorted by that group's realized edge count -> exact ELL
  round prefixes).
- ap_gather gathers message streams (rounds, zero-slot padded); DVE adds
  accumulate round prefixes into acc [128, 12544].
- A second small ap_gather canonicalizes each group's acc into the shared
  local-slot order; a PE matmul with a 0/1 selection matrix sums the 8
  groups; epilogue (own-chunk add, dinv scale, bias, relu, W2, W_lin)
  runs feature-major in 448-wide pieces on PE/DVE/ACT.
- dinv (deg^-1/2, 0 on padded slots) is precomputed on host as [16,12544].
"""
import sys
sys.path.insert(0, "/opt/trn_rl_repo")
import numpy as np

N_NODES = 100000
N_EDGES = 3200000
D_IN = 128
H = 16
CORES = 8
LOC = 12544          # padded local slots per core (12500 real)
REAL = 12500
ZERO_SLOT = 12500    # any padded local slot: y value is 0 there
CALL = 1792          # idxs per ap_gather call (= 4 * PIECE)
PIECE = 448          # matmul moving width (LOC = 28 * 448)


# ---------------------------------------------------------------- host prep
def _ceil16(x):
    return -(-x // 16) * 16


def host_prep(x, edge_index):
    src = edge_index[0].astype(np.int64)
    dst = edge_index[1].astype(np.int64)

    # degree INCLUDES self-loops (reference adds them), even though the
    # self-loop messages themselves are handled outside the gather.
    deg = np.bincount(dst, minlength=N_NODES) + 1
    perm = np.argsort(-deg, kind="stable")
    rank = np.empty(N_NODES, np.int64)
    rank[perm] = np.arange(N_NODES)

    s_dst = rank[dst]
    s_src = rank[src]
    core_e = s_dst % CORES
    dloc_e = s_dst // CORES
    g_e = (s_src % CORES).astype(np.int64)
    sloc_e = s_src // CORES

    # per (core k, group g): counts, private order, sorted edge lists
    per = {}
    Lmax = {}
    Rglob = 0
    for k in range(CORES):
        mk = core_e == k
        for g in range(CORES):
            m = mk & (g_e == g)
            dl = dloc_e[m]
            sl = sloc_e[m]
            cnt = np.bincount(dl, minlength=LOC)
            order = np.argsort(-cnt, kind="stable")       # group pos -> dloc
            pos_of = np.empty(LOC, np.int64)
            pos_of[order] = np.arange(LOC)
            o = np.argsort(dl, kind="stable")
            sl_sorted = sl[o]
            starts = np.zeros(LOC + 1, np.int64)
            starts[1:] = np.cumsum(cnt)
            R = int(cnt.max())
            Rglob = max(Rglob, R)
            per[(k, g)] = (cnt, order, pos_of, sl_sorted, starts)
            cnt_sorted = cnt[order]
            for r in range(1, R + 1):
                nz = np.nonzero(cnt_sorted >= r)[0]
                L = int(nz[-1]) + 1 if nz.size else 0
                Lmax[r] = max(Lmax.get(r, 0), L)

    L16 = [_ceil16(Lmax[r]) for r in range(1, Rglob + 1)]
    offs = np.concatenate([[0], np.cumsum(L16)]).astype(np.int64)
    TOT = int(offs[-1])

    # calls and add-segments (shared structure)
    n_call = -(-TOT // CALL)
    call_len = [min(CALL, TOT - c * CALL) for c in range(n_call)]
    segments = []  # (call, dest_off, acc_off, length)
    for r in range(Rglob):
        a, b = int(offs[r]), int(offs[r + 1])
        p = a
        while p < b:
            c = p // CALL
            e = min(b, (c + 1) * CALL)
            segments.append((c, p - c * CALL, p - a, e - p))
            p = e

    # per-core idx tensors
    def wrap16(flat):
        n = flat.size
        return flat.reshape(n // 16, 16).T

    IDX = np.full((CORES, 128, TOT // 16), ZERO_SLOT, np.int16)
    IDXC = np.zeros((CORES, 128, LOC // 16), np.int16)
    for k in range(CORES):
        for g in range(CORES):
            cnt, order, pos_of, sl_sorted, starts = per[(k, g)]
            stream = np.full(TOT, ZERO_SLOT, np.int64)
            for r in range(1, Rglob + 1):
                a = int(offs[r - 1])
                L = L16[r - 1]
                d_arr = order[:L]
                valid = cnt[d_arr] >= r
                pos = np.clip(starts[d_arr] + r - 1, 0, sl_sorted.size - 1)
                vals = np.where(valid, sl_sorted[pos] if sl_sorted.size else 0,
                                ZERO_SLOT)
                stream[a : a + L] = vals
            IDX[k, 16 * g : 16 * g + 16, :] = wrap16(stream)
            IDXC[k, 16 * g : 16 * g + 16, :] = wrap16(pos_of)

    # per-core x.T and dinv
    XT = np.zeros((CORES, 128, LOC), np.float32)
    DINV = np.zeros((CORES, 16, LOC), np.float32)
    node_of = np.zeros((CORES, REAL), np.int64)
    dinv_all = 1.0 / np.sqrt(deg.astype(np.float64))
    for k in range(CORES):
        nodes = perm[k::CORES]           # local j -> node
        node_of[k] = nodes
        XT[k, :, :REAL] = x[nodes].T
        DINV[k, :, :REAL] = dinv_all[nodes][None, :].astype(np.float32)

    SMERGE = np.zeros((128, 16), np.float32)
    for g in range(CORES):
        for f in range(16):
            SMERGE[16 * g + f, f] = 1.0
    # per-core selector for the self-loop term: pick the core's own chunk
    # (partitions 16k..16k+15 of tab, already in canonical slot order)
    SMK = np.zeros((CORES, 128, 16), np.float32)
    for k in range(CORES):
        for f in range(16):
            SMK[k, 16 * k + f, f] = 1.0

    struct = dict(Rglob=Rglob, L16=L16, TOT=TOT, n_call=n_call,
                  call_len=call_len, segments=segments)
    return struct, IDX, IDXC, XT, DINV, SMERGE, SMK, node_of


# ---------------------------------------------------------------- device build
def build_nc(struct):
    import concourse.bass as bass
    import concourse.bacc as bacc
    import concourse.mybir as mybir
    import concourse.tile as tile

    f32 = mybir.dt.float32
    TOT, n_call = struct["TOT"], struct["n_call"]
    call_len, segments = struct["call_len"], struct["segments"]

    nc = bacc.Bacc(None, target_bir_lowering=False)
    t_xt = nc.dram_tensor("xt", [128, LOC], f32, kind="ExternalInput")
    t_idx = nc.dram_tensor("idx", [128, TOT // 16], mybir.dt.int16,
                           kind="ExternalInput")
    t_idxc = nc.dram_tensor("idxc", [128, LOC // 16], mybir.dt.int16,
                            kind="ExternalInput")
    t_dinv = nc.dram_tensor("dinv", [16, LOC], f32, kind="ExternalInput")
    t_w1 = nc.dram_tensor("w1", [128, H], f32, kind="ExternalInput")
    t_w2 = nc.dram_tensor("w2", [H, H], f32, kind="ExternalInput")
    t_wl = nc.dram_tensor("wl", [H, 1], f32, kind="ExternalInput")
    t_b1 = nc.dram_tensor("b1", [H, 1], f32, kind="ExternalInput")
    t_b2 = nc.dram_tensor("b2", [H, 1], f32, kind="ExternalInput")
    t_sm = nc.dram_tensor("sm", [128, H], f32, kind="ExternalInput")
    t_smk = nc.dram_tensor("smk", [128, H], f32, kind="ExternalInput")
    t_out = nc.dram_tensor("out", [1, LOC], f32, kind="ExternalOutput")
    # b_lin is 0 in the reference; fold as constant 0 (skip).

    n_piece = LOC // PIECE
    assert LOC % PIECE == 0

    with tile.TileContext(nc) as tc:
        with (
            tc.tile_pool(name="sbuf", bufs=1) as pool,
            tc.tile_pool(name="io", bufs=3) as iop,
            tc.tile_pool(name="io2", bufs=6) as iop2,
            tc.tile_pool(name="psum", bufs=2, space="PSUM") as pp,
            tc.tile_pool(name="dram", bufs=1, space="DRAM") as dram,
        ):
            # persistent tiles
            idxt = pool.tile([128, TOT // 16], mybir.dt.int16, name="idxt")
            idxct = pool.tile([128, LOC // 16], mybir.dt.int16, name="idxct")
            tab = pool.tile([128, LOC], f32, name="tab")
            acc = pool.tile([128, LOC], f32, name="acc")
            w1t = pool.tile([128, H], f32, name="w1t")
            w2t = pool.tile([H, H], f32, name="w2t")
            wlt = pool.tile([H, 1], f32, name="wlt")
            b1t = pool.tile([H, 1], f32, name="b1t")
            b2t = pool.tile([H, 1], f32, name="b2t")
            smt = pool.tile([128, H], f32, name="smt")
            smkt = pool.tile([128, H], f32, name="smkt")

            nc.sync.dma_start(out=idxt[:], in_=t_idx[:, :])
            nc.sync.dma_start(out=idxct[:], in_=t_idxc[:, :])
            nc.sync.dma_start(out=w1t[:], in_=t_w1[:, :])
            nc.sync.dma_start(out=w2t[:], in_=t_w2[:, :])
            nc.sync.dma_start(out=wlt[:], in_=t_wl[:, :])
            nc.sync.dma_start(out=b1t[:], in_=t_b1[:, :])
            nc.sync.dma_start(out=b2t[:], in_=t_b2[:, :])
            nc.sync.dma_start(out=smt[:], in_=t_sm[:, :])
            nc.sync.dma_start(out=smkt[:], in_=t_smk[:, :])

            HALF = 14 * PIECE            # 6272: AllGather split point
            ytab_full = []
            for layer in range(2):
                bounce_a = dram.tile([16, HALF], f32, tag=f"bna{layer}",
                                     name=f"bna{layer}")
                bounce_b = dram.tile([16, LOC - HALF], f32,
                                     tag=f"bnb{layer}", name=f"bnb{layer}")
                full_a = nc.dram_tensor(f"full{layer}a", [CORES, 16, HALF],
                                        f32, addr_space="Shared")
                full_b = nc.dram_tensor(f"full{layer}b", [CORES, 16,
                                        LOC - HALF], f32, addr_space="Shared")
                ytab_full.append((bounce_a, bounce_b, full_a, full_b))

            def bounce_slice(layer, off):
                # piece destination within the layer's split bounce pair
                bounce_a, bounce_b = ytab_full[layer][:2]
                if off < HALF:
                    return bounce_a[:, off : off + PIECE]
                return bounce_b[:, off - HALF : off - HALF + PIECE]

            def ag_half(layer, half):
                # the collective is a non-blocking issue on POOL (transfer
                # runs on the CC DMA); splitting lets the first half's
                # transfer overlap the compute producing the second half
                bounce_a, bounce_b, full_a, full_b = ytab_full[layer]
                if half == 0:
                    ins, outs = bounce_a[:], full_a.ap()
                else:
                    ins, outs = bounce_b[:], full_b.ap()
                nc.gpsimd.collective_compute(
                    "AllGather", mybir.AluOpType.bypass,
                    replica_groups=[list(range(CORES))],
                    ins=[ins.opt()], outs=[outs.opt()])

            def load_tab(layer):
                full_a, full_b = ytab_full[layer][2:]
                for g in range(CORES):
                    nc.sync.dma_start(out=tab[16 * g : 16 * g + 16, :HALF],
                                      in_=full_a[g, :, :])
                    nc.sync.dma_start(out=tab[16 * g : 16 * g + 16, HALF:],
                                      in_=full_b[g, :, :])

            CH = 2 * PIECE               # 896: xT/dinv load chunk

            def build_table_layer0():
                # y1 = dinv * (W1.T @ xT), chunked loads, piece matmuls
                for c in range(LOC // CH):
                    base = c * CH
                    xin = iop2.tile([128, CH], f32, tag="xin")
                    nc.sync.dma_start(out=xin[:],
                                      in_=t_xt[:, base : base + CH])
                    dv = iop2.tile([16, CH], f32, tag="dvb")
                    nc.sync.dma_start(out=dv[:],
                                      in_=t_dinv[:, base : base + CH])
                    for q in range(CH // PIECE):
                        a = base + q * PIECE
                        ps = pp.tile([16, PIECE], f32, tag="ps")
                        nc.tensor.matmul(
                            out=ps[:], lhsT=w1t[:],
                            rhs=xin[:, q * PIECE : (q + 1) * PIECE],
                            start=True, stop=True)
                        yp = iop.tile([16, PIECE], f32, tag="ep")
                        nc.vector.tensor_mul(
                            out=yp[:], in0=ps[:],
                            in1=dv[:, q * PIECE : (q + 1) * PIECE])
                        nc.sync.dma_start(out=bounce_slice(0, a), in_=yp[:])
                        if a + PIECE == HALF:
                            ag_half(0, 0)
                ag_half(0, 1)

            def gather_accumulate():
                nc.vector.memset(acc[:], 0.0)
                for c in range(n_call):
                    ln = call_len[c]
                    d = iop.tile([128, CALL], f32, tag="gd")
                    nc.gpsimd.ap_gather(
                        d[:, :ln], tab[:],
                        idxt[:, c * (CALL // 16) : c * (CALL // 16) + ln // 16],
                        channels=128, num_elems=LOC, d=1, num_idxs=ln)
                    for (cc, doff, aoff, ln2) in segments:
                        if cc != c:
                            continue
                        nc.vector.tensor_add(
                            out=acc[:, aoff : aoff + ln2],
                            in0=acc[:, aoff : aoff + ln2],
                            in1=d[:, doff : doff + ln2])

            def canonicalize_and_epilogue(layer):
                n_cc = LOC // CALL + (1 if LOC % CALL else 0)
                for c in range(n_cc):
                    a = c * CALL
                    b = min(LOC, a + CALL)
                    w = b - a
                    cd = iop.tile([128, CALL], f32, tag="gd")
                    nc.gpsimd.ap_gather(
                        cd[:, :w], acc[:],
                        idxct[:, a // 16 : b // 16],
                        channels=128, num_elems=LOC, d=1, num_idxs=w)
                    dv = iop.tile([16, CALL], f32, tag="dvc")
                    nc.sync.dma_start(out=dv[:, :w], in_=t_dinv[:, a:b])
                    for q in range(w // PIECE):
                        off = a + q * PIECE
                        ps = pp.tile([16, PIECE], f32, tag="ps")
                        # sum the 8 group partials, then accumulate the
                        # self-loop term (core's own tab chunk) in PSUM
                        nc.tensor.matmul(out=ps[:], lhsT=smt[:],
                                         rhs=cd[:, q * PIECE : (q + 1) * PIECE],
                                         start=True, stop=False)
                        nc.tensor.matmul(out=ps[:], lhsT=smkt[:],
                                         rhs=tab[:, off : off + PIECE],
                                         start=False, stop=True)
                        v = iop.tile([16, PIECE], f32, tag="ep")
                        nc.vector.tensor_mul(
                            out=v[:], in0=ps[:],
                            in1=dv[:, q * PIECE : q * PIECE + PIECE])
                        if layer == 0:
                            # y2 = dinv * relu(v + b1) -> bounce1
                            h = iop.tile([16, PIECE], f32, tag="ep")
                            nc.scalar.activation(
                                out=h[:], in_=v[:],
                                func=mybir.ActivationFunctionType.Relu,
                                bias=b1t[:])
                            y2 = iop.tile([16, PIECE], f32, tag="ep")
                            nc.vector.tensor_mul(
                                out=y2[:], in0=h[:],
                                in1=dv[:, q * PIECE : q * PIECE + PIECE])
                            nc.sync.dma_start(
                                out=bounce_slice(1, off), in_=y2[:])
                            if off + PIECE == HALF:
                                ag_half(1, 0)
                        else:
                            # z = W2.T @ v ; h2 = relu(z + b2); o = Wl.T @ h2
                            ps2 = pp.tile([16, PIECE], f32, tag="ps2")
                            nc.tensor.matmul(out=ps2[:], lhsT=w2t[:],
                                             rhs=v[:], start=True, stop=True)
                            h2 = iop.tile([16, PIECE], f32, tag="ep")
                            nc.scalar.activation(
                                out=h2[:], in_=ps2[:],
                                func=mybir.ActivationFunctionType.Relu,
                                bias=b2t[:])
                            ps3 = pp.tile([1, PIECE], f32, tag="ps3")
                            nc.tensor.matmul(out=ps3[:], lhsT=wlt[:],
                                             rhs=h2[:], start=True, stop=True)
                            ob = iop.tile([1, PIECE], f32, tag="ep")
                            nc.vector.tensor_copy(out=ob[:], in_=ps3[:])
                            nc.sync.dma_start(
                                out=t_out[:, off : off + PIECE], in_=ob[:])

            build_table_layer0()
            load_tab(0)
            gather_accumulate()
            canonicalize_and_epilogue(0)
            ag_half(1, 1)
            load_tab(1)
            gather_accumulate()
            canonicalize_and_epilogue(1)

    nc.finalize()
    return nc


# ---------------------------------------------------------------- runner
class _Runner:
    def __init__(self, nc, n_cores):
        import jax
        import numpy as _np
        from jax.sharding import Mesh, PartitionSpec, NamedSharding
        from jax.experimental.shard_map import shard_map
        import concourse.mybir as mybir
        from concourse.bass2jax import (
            _bass_exec_p, install_neuronx_cc_hook, partition_id_tensor)

        install_neuronx_cc_hook()
        self.nc = nc
        self.n_cores = n_cores
        partition_name = (nc.partition_id_tensor.name
                          if nc.partition_id_tensor else None)
        in_names, out_names, out_avals, zero_outs = [], [], [], []
        for alloc in nc.m.functions[0].allocations:
            if not isinstance(alloc, mybir.MemoryLocationSet):
                continue
            name = alloc.memorylocations[0].name
            if alloc.kind == "ExternalInput":
                if name != partition_name:
                    in_names.append(name)
            elif alloc.kind == "ExternalOutput":
                shape = tuple(alloc.tensor_shape)
                dtype = mybir.dt.np(alloc.dtype)
                out_names.append(name)
                out_avals.append(jax.core.ShapedArray(shape, dtype))
                zero_outs.append(_np.zeros(shape, dtype))
        self.in_names, self.out_names = in_names, out_names
        self.out_avals, self.zero_outs = out_avals, zero_outs
        n_params, n_outs = len(in_names), len(out_avals)
        all_in = in_names + out_names
        if partition_name is not None:
            all_in.append(partition_name)
        donate = tuple(range(n_params, n_params + n_outs))

        def _body(*args):
            operands = list(args)
            if partition_name is not None:
                operands.append(partition_id_tensor())
            return tuple(_bass_exec_p.bind(
                *operands, out_avals=tuple(out_avals),
                in_names=tuple(all_in), out_names=tuple(out_names),
                lowering_input_output_aliases=(),
                sim_require_finite=True, sim_require_nnan=True, nc=nc))

        devices = jax.devices()[:n_cores]
        mesh = Mesh(_np.asarray(devices), ("core",))
        self._sharding = NamedSharding(mesh, PartitionSpec("core"))
        in_specs = (PartitionSpec("core"),) * (n_params + n_outs)
        out_specs = (PartitionSpec("core"),) * len(out_names)
        self._fn = jax.jit(
            shard_map(_body, mesh=mesh, in_specs=in_specs,
                      out_specs=out_specs, check_rep=False),
            donate_argnums=donate, keep_unused=True)

        # device-side zero buffers for the donated outputs: generated on
        # device each call so repeats never pay H2D for them.
        import jax.numpy as jnp
        zero_shapes = [(n_cores * z.shape[0], *z.shape[1:])
                       for z in self.zero_outs]
        zero_dtypes = [z.dtype for z in self.zero_outs]

        def _mk_zeros():
            return tuple(jnp.zeros(s, d) for s, d in
                         zip(zero_shapes, zero_dtypes))

        self._mk_zeros = jax.jit(
            _mk_zeros, out_shardings=tuple([self._sharding] * n_outs))
        # cache of device-resident concatenated inputs (keyed on in_maps id)
        self._dev_key = None
        self._dev_in = None
        # previous call's output buffers, recycled as the next call's
        # donated output arguments (the kernel writes every element of
        # each output, so stale contents are harmless)
        self._recycle = None

    def _device_inputs(self, in_maps):
        import jax
        import numpy as _np
        key = id(in_maps)
        if self._dev_key == key and self._dev_in is not None:
            return self._dev_in
        n = self.n_cores
        per_core = [[_np.asarray(m[name]) for name in self.in_names]
                    for m in in_maps]
        concat_in = [
            _np.concatenate([per_core[c][i] for c in range(n)], axis=0)
            for i in range(len(self.in_names))]
        self._dev_in = [jax.device_put(a, self._sharding) for a in concat_in]
        self._dev_key = key
        return self._dev_in

    def __call__(self, in_maps):
        import numpy as _np
        n = self.n_cores
        dev_in = self._device_inputs(in_maps)
        donated = self._recycle if self._recycle is not None \
            else self._mk_zeros()
        outs = self._fn(*dev_in, *donated)
        out_arrs = [_np.asarray(a) for a in outs]
        self._recycle = outs
        return [
            {name: out_arrs[i].reshape(n, *self.out_avals[i].shape)[c]
             for i, name in enumerate(self.out_names)}
            for c in range(n)]


_CACHE = {}


def kernel(x, edge_index, W1, b1, W2, b2, W_lin, b_lin):
    x = np.asarray(x, np.float32)
    edge_index = np.asarray(edge_index)
    struct, IDX, IDXC, XT, DINV, SMERGE, SMK, node_of = host_prep(
        x, edge_index)

    key = repr(sorted(struct.items()))
    if key not in _CACHE:
        nc = build_nc(struct)
        _CACHE[key] = _Runner(nc, CORES)
    runner = _CACHE[key]

    in_maps = []
    for k in range(CORES):
        in_maps.append({
            "xt": XT[k], "idx": IDX[k], "idxc": IDXC[k], "dinv": DINV[k],
            "w1": np.asarray(W1, np.float32),
            "w2": np.asarray(W2, np.float32),
            "wl": np.asarray(W_lin, np.float32),
            "b1": np.asarray(b1, np.float32).reshape(H, 1),
            "b2": np.asarray(b2, np.float32).reshape(H, 1),
            "sm": SMERGE,
            "smk": SMK[k],
        })
    res = runner(in_maps)
    out = np.zeros(N_NODES, np.float32)
    blin = float(np.asarray(b_lin).reshape(-1)[0])
    for k in range(CORES):
        out[node_of[k]] = res[k]["out"][0, :REAL] + blin
    kernel.last_runner = runner
    kernel.last_in_maps = in_maps
    return out
